# revision 14
# baseline (speedup 1.0000x reference)
"""Batched MoE (top-2, 8 experts) on 8 Trainium2 NeuronCores.

Strategy: expert-parallel — core e owns expert e's weights (w1/w2/w3) and
processes the tokens routed to it. Routing (sort by expert / capacity
padding) and the combine (weighting by gate prob + scatter-add over top-k)
are cheap O(tokens) index ops done on host; all matmul FLOPs run on device.

Device dataflow per core (capacity C columns, zero-padded):
    xt  = X_e^T               [1024, C]   (d on partitions)
    GT  = w1^T @ xt           [4096, C]   lhsT = w1 tiles (natural layout)
    VT  = w2^T @ xt           [4096, C]
    HT  = silu(GT) * VT       [4096, C]
    OT  = w3^T @ HT           [1024, C]   lhsT = w3 tiles (natural layout)

Everything is bf16 (PSUM/OT accumulation in fp32): same PE rate as
float32r (1 col/cycle) but half the HBM traffic, and bf16 LDWEIGHTS (FWL)
hides under the previous matmul's streaming. Matmul loops are
column-chunk-outer / k-inner so consecutive matmuls always carry different
weight tiles (a same-cell weight reload stalls the PE).

All tensors ride ONE merged DMA per (matrix, chunk) using a
"(k p) f -> p k f" partition-major rearrange: ~1MB transfers keep the DMA
queue near line rate, and the short first/last chunks (f=256) shrink the
DMA critical path at the ramp (so HAM un-throttles early) and the
phase-B/store drain at the tail. OT is stored as bf16; the final
accumulation's DVE add does the fp32->bf16 cast for free.
"""

import numpy as np

N_EXPERTS = 8
D_MODEL = 1024
D_FF = 4096
# first/last chunks short: fast ramp, short tail
CHUNKS = [256, 512, 512, 512, 512, 512, 512, 512, 256]
KT = D_MODEL // 128     # 8 k-tiles (contraction d)
MT = D_MODEL // 128     # 8 output d-tiles

_program_cache = {}


def _col_chunks(C):
    """Split C columns into <=512 pieces (PSUM bank = 512 fp32)."""
    if C <= 512:
        return [(0, C)]
    n = (C + 511) // 512
    base = C // n
    rem = C - base * n
    out = []
    off = 0
    for i in range(n):
        sz = base + (1 if i < rem else 0)
        out.append((off, sz))
        off += sz
    return out


def _build_program(C):
    import concourse.bacc as bacc
    import concourse.mybir as mybir
    from concourse.tile import TileContext

    BF16 = mybir.dt.bfloat16
    F32 = mybir.dt.float32
    SILU = mybir.ActivationFunctionType.Silu
    ccs = _col_chunks(C)

    nc = bacc.Bacc()
    # host pre-rearranges every tensor into partition-major, chunk-contiguous
    # layout so each chunk load is ONE fully-contiguous multi-KB-per-partition
    # DMA (line rate ~390GB/s vs ~170GB/s for sub-KB strided rows)
    xt_d = nc.declare_dram_parameter("xt", [128, KT * C], BF16, isOutput=False)
    w1_d = nc.declare_dram_parameter("w1", [128, KT * D_FF], BF16, isOutput=False)
    w2_d = nc.declare_dram_parameter("w2", [128, KT * D_FF], BF16, isOutput=False)
    w3_d = nc.declare_dram_parameter(
        "w3", [128, (D_FF // 128) * D_MODEL], BF16, isOutput=False
    )
    ot_d = nc.declare_dram_parameter("ot", [D_MODEL, C], BF16, isOutput=True)

    xt_r = xt_d.rearrange("p (k c) -> p k c", k=KT)
    ot_r = ot_d.rearrange("(m p) c -> m p c", p=128)

    NCH = len(CHUNKS)
    f_offs = [sum(CHUNKS[:i]) for i in range(NCH)]

    with TileContext(nc) as tc:
        with (
            tc.tile_pool(name="xtp", bufs=1) as xt_pool,
            tc.tile_pool(name="w12", bufs=2) as w12_pool,
            tc.tile_pool(name="w3p", bufs=2) as w3_pool,
            tc.tile_pool(name="htp", bufs=2) as ht_pool,
            tc.tile_pool(name="otp", bufs=1) as ot_pool,
            tc.tile_pool(name="tmp", bufs=4) as tmp_pool,
            tc.tile_pool(name="pg", bufs=2, space="PSUM") as pg_pool,
            tc.tile_pool(name="pv", bufs=2, space="PSUM") as pv_pool,
            tc.tile_pool(name="po", bufs=4, space="PSUM") as po_pool,
        ):
            ot_sb = [
                ot_pool.tile([128, C], F32, tag=f"ot{m}", name=f"ot{m}")
                for m in range(MT)
            ]
            ot_bf = [
                ot_pool.tile([128, C], BF16, tag=f"otb{m}", name=f"otb{m}")
                for m in range(MT)
            ]

            # HAM warm-up: a dependency-free matmul burst fills the
            # ~5us the PE would otherwise idle waiting for the first weight
            # DMAs, so the clock gate is at 2.4GHz when real work arrives.
            dmy = tmp_pool.tile([128, 264], BF16, tag="dmy", name="dmy")
            nc.vector.memset(dmy[:], 0.0)
            pw = po_pool.tile(
                [128, 264], F32, tag="po", name="pw", padded_shape=[128, 512]
            )
            for i in range(6):
                nc.tensor.matmul(
                    out=pw[:],
                    lhsT=dmy[:, 128 * (i % 2) : 128 * (i % 2) + 128],
                    rhs=dmy[:],
                    start=(i == 0),
                    stop=(i == 5),
                )

            # xt streams in two k-halves (each contiguous) so chunk 0's
            # matmuls can start on k0-3 while k4-7 is still in flight
            xt_sb = xt_pool.tile([128, KT, C], BF16, tag="xt", name="xt")
            KH = KT // 2
            nc.sync.dma_start(out=xt_sb[:, 0:KH, :], in_=xt_r[:, 0:KH, :])

            def load_w1(ch):
                f0, fc = f_offs[ch], CHUNKS[ch]
                t1 = w12_pool.tile([128, KT, fc], BF16, tag="w1c", name=f"w1c{ch}")
                nc.sync.dma_start(
                    out=t1[:], in_=w1_d[:, KT * f0 : KT * (f0 + fc)]
                )
                return t1

            def load_w2(ch):
                f0, fc = f_offs[ch], CHUNKS[ch]
                t2 = w12_pool.tile([128, KT, fc], BF16, tag="w2c", name=f"w2c{ch}")
                nc.sync.dma_start(
                    out=t2[:], in_=w2_d[:, KT * f0 : KT * (f0 + fc)]
                )
                return t2

            def load_w3(ch):
                f0, fc = f_offs[ch], CHUNKS[ch]
                jt = fc // 128
                t3 = w3_pool.tile(
                    [128, jt, D_MODEL], BF16, tag="w3c", name=f"w3c{ch}"
                )
                j0 = f0 // 128
                nc.sync.dma_start(
                    out=t3[:], in_=w3_d[:, D_MODEL * j0 : D_MODEL * (j0 + jt)]
                )
                return t3

            def phase_a(ch, w1c, w2c):
                """GT/VT matmuls + silu*mul epilogue -> HT tiles for a chunk.

                For chunk 0 both column-chunks' G groups run before the V
                groups so the PE has w1-only work while w2's DMA is still in
                flight at the ramp."""
                jt = CHUNKS[ch] // 128
                hts = []
                for jj in range(jt):
                    ht_t = ht_pool.tile(
                        [128, C], BF16, tag=f"ht{jj % 4}", name=f"ht{jj}"
                    )
                    js = slice(jj * 128, (jj + 1) * 128)
                    pgs, pvs = [], []
                    for c0, cl in ccs:
                        pgs.append(pg_pool.tile(
                            [128, cl], F32, tag="pg", name="pg",
                            padded_shape=[128, 512],
                        ))
                        pvs.append(pv_pool.tile(
                            [128, cl], F32, tag="pv", name="pv",
                            padded_shape=[128, 512],
                        ))

                    def g_group(i):
                        c0, cl = ccs[i]
                        for k in range(KT):
                            nc.tensor.matmul(
                                out=pgs[i][:],
                                lhsT=w1c[:, k, js],
                                rhs=xt_sb[:, k, c0 : c0 + cl],
                                start=(k == 0),
                                stop=(k == KT - 1),
                            )

                    def v_group(i):
                        c0, cl = ccs[i]
                        for k in range(KT):
                            nc.tensor.matmul(
                                out=pvs[i][:],
                                lhsT=w2c[:, k, js],
                                rhs=xt_sb[:, k, c0 : c0 + cl],
                                start=(k == 0),
                                stop=(k == KT - 1),
                            )

                    def epilogue(i):
                        c0, cl = ccs[i]
                        cs = slice(c0, c0 + cl)
                        st = tmp_pool.tile([128, cl], F32, tag="silu", name="st")
                        nc.scalar.activation(st[:], pgs[i][:], SILU)
                        nc.vector.tensor_mul(
                            out=ht_t[:, cs], in0=st[:], in1=pvs[i][:]
                        )

                    for i in range(len(ccs)):
                        g_group(i)
                        v_group(i)
                        epilogue(i)
                    hts.append(ht_t)
                return hts

            def phase_a0(w1c, w2c):
                """Chunk 0, ramp-ordered: process column-chunk 0 entirely
                (G for all jj, then V+epilogue) before touching column-chunk
                1, matching the DMA arrival order xt_h0, w1c0, w2c0, xt_h1."""
                jt = CHUNKS[0] // 128
                hts = [
                    ht_pool.tile([128, C], BF16, tag=f"ht{jj % 4}", name=f"ht{jj}")
                    for jj in range(jt)
                ]
                for i, (c0, cl) in enumerate(ccs):
                    cs = slice(c0, c0 + cl)
                    pgs = []
                    for jj in range(jt):
                        js = slice(jj * 128, (jj + 1) * 128)
                        pg = pg_pool.tile(
                            [128, cl], F32, tag="pg", name="pg",
                            padded_shape=[128, 512],
                        )
                        for k in range(KT):
                            nc.tensor.matmul(
                                out=pg[:],
                                lhsT=w1c[:, k, js],
                                rhs=xt_sb[:, k, cs],
                                start=(k == 0),
                                stop=(k == KT - 1),
                            )
                        pgs.append(pg)
                    for jj in range(jt):
                        js = slice(jj * 128, (jj + 1) * 128)
                        pv = pv_pool.tile(
                            [128, cl], F32, tag="pv", name="pv",
                            padded_shape=[128, 512],
                        )
                        for k in range(KT):
                            nc.tensor.matmul(
                                out=pv[:],
                                lhsT=w2c[:, k, js],
                                rhs=xt_sb[:, k, cs],
                                start=(k == 0),
                                stop=(k == KT - 1),
                            )
                        st = tmp_pool.tile([128, cl], F32, tag="silu", name="st")
                        nc.scalar.activation(st[:], pgs[jj][:], SILU)
                        nc.vector.tensor_mul(
                            out=hts[jj][:, cs], in0=st[:], in1=pv[:]
                        )
                return hts

            def phase_b_m(ch, w3c, hts, m):
                """OT partial accumulation for one output d-tile of a chunk."""
                jt = len(hts)
                ms = slice(m * 128, (m + 1) * 128)
                for c0, cl in ccs:
                    cs = slice(c0, c0 + cl)
                    po = po_pool.tile(
                        [128, cl], F32, tag="po", name="po",
                        padded_shape=[128, 512],
                    )
                    for jj in range(jt):
                        nc.tensor.matmul(
                            out=po[:],
                            lhsT=w3c[:, jj, ms],
                            rhs=hts[jj][:, cs],
                            start=(jj == 0),
                            stop=(jj == jt - 1),
                        )
                    if ch == 0:
                        nc.vector.tensor_copy(out=ot_sb[m][:, cs], in_=po[:])
                    elif ch == NCH - 1:
                        # final accumulation writes the bf16 store tile (DVE
                        # converts on output) so the OT store DMA is half-size
                        nc.vector.tensor_add(
                            out=ot_bf[m][:, cs], in0=ot_sb[m][:, cs], in1=po[:]
                        )
                    else:
                        nc.vector.tensor_add(
                            out=ot_sb[m][:, cs], in0=ot_sb[m][:, cs], in1=po[:]
                        )

            def phase_b(ch, w3c, hts):
                for m in range(MT):
                    phase_b_m(ch, w3c, hts, m)

            # software pipeline: B(ch-1) issues after A(ch) so phase B never
            # stalls the PE on the ACT/DVE epilogue producing its HT input.
            # DMAs are emitted in exact PE consumption order. Only the last
            # (short) B pass drains at the end, interleaved m-wise with the
            # (bf16, half-size) OT store DMAs.
            w1c = load_w1(0)
            nc.sync.dma_start(out=xt_sb[:, KH:KT, :], in_=xt_r[:, KH:KT, :])
            w2c = load_w2(0)
            hts_prev = phase_a0(w1c, w2c)
            w3_prev = None
            hts_m2 = None
            for ch in range(1, NCH):
                w1c, w2c = load_w1(ch), load_w2(ch)
                w3_prev = load_w3(ch - 1)
                hts = phase_a(ch, w1c, w2c)
                if ch < NCH - 1:
                    phase_b(ch - 1, w3_prev, hts_prev)
                else:
                    hts_m2 = hts_prev  # B(NCH-2) merges into the final pass
                hts_prev = hts
            w3_last = load_w3(NCH - 1)
            # final pass: one PSUM group accumulates BOTH remaining chunks'
            # contributions per (m, column-chunk) -> half the endgame DVE adds
            for m in range(MT):
                ms = slice(m * 128, (m + 1) * 128)
                for c0, cl in ccs:
                    cs = slice(c0, c0 + cl)
                    po = po_pool.tile(
                        [128, cl], F32, tag="po", name="po",
                        padded_shape=[128, 512],
                    )
                    groups = [(w3_prev, hts_m2), (w3_last, hts_prev)]
                    n_tot = sum(len(h) for _, h in groups)
                    idx = 0
                    for w3c, hts_g in groups:
                        for jj in range(len(hts_g)):
                            nc.tensor.matmul(
                                out=po[:],
                                lhsT=w3c[:, jj, ms],
                                rhs=hts_g[jj][:, cs],
                                start=(idx == 0),
                                stop=(idx == n_tot - 1),
                            )
                            idx += 1
                    nc.vector.tensor_add(
                        out=ot_bf[m][:, cs], in0=ot_sb[m][:, cs], in1=po[:]
                    )
                nc.sync.dma_start(out=ot_r[m], in_=ot_bf[m][:])

    nc.compile()
    return nc


def _get_program(C):
    if C not in _program_cache:
        _program_cache[C] = _build_program(C)
    return _program_cache[C]


def _run(nc, in_maps, trace=False):
    import time

    from concourse.bass_utils import run_bass_kernel_spmd

    last = None
    for attempt in range(4):
        try:
            return run_bass_kernel_spmd(
                nc, in_maps, list(range(N_EXPERTS)), trace=trace
            )
        except Exception as e:  # stale device state from a prior crashed run
            last = e
            time.sleep(10 * (attempt + 1))
            try:  # poke the runtime with a trivial op to clear/verify state
                import jax
                import jax.numpy as jnp

                jnp.add(jnp.ones((8, 8)), 1.0).block_until_ready()
            except Exception:
                pass
    raise last


def kernel(x, expert_indices, expert_weights, w1, w2, w3, _trace=False):
    import ml_dtypes

    BF = ml_dtypes.bfloat16
    x = np.ascontiguousarray(np.asarray(x, dtype=np.float32))
    expert_indices = np.asarray(expert_indices)
    expert_weights = np.asarray(expert_weights, dtype=np.float32)
    w1 = np.asarray(w1, dtype=np.float32).astype(BF)
    w2 = np.asarray(w2, dtype=np.float32).astype(BF)
    w3 = np.asarray(w3, dtype=np.float32).astype(BF)

    n_tokens, d_model = x.shape
    top_k = expert_indices.shape[1]
    n_experts = w1.shape[0]
    A = n_tokens * top_k

    flat_e = expert_indices.reshape(-1).astype(np.int64)
    flat_w = expert_weights.reshape(-1)
    tok_idx = np.repeat(np.arange(n_tokens), top_k)
    order = np.argsort(flat_e, kind="stable")
    s_tok = tok_idx[order]
    s_w = flat_w[order]
    counts = np.bincount(flat_e, minlength=n_experts)
    starts = np.concatenate([[0], np.cumsum(counts)[:-1]])

    C = int(counts.max())
    C = max(256, -(-C // 8) * 8)  # round up to multiple of 8 (16B bf16 rows)

    xb = x.astype(BF)
    KT_, f_offs = d_model // 128, np.cumsum([0] + CHUNKS[:-1])

    def chunked_w12(w):  # [1024, 4096] -> [128, KT*4096] chunk-contiguous
        t = w.reshape(KT_, 128, D_FF).transpose(1, 0, 2)
        parts = [
            np.ascontiguousarray(t[:, :, f0 : f0 + fc]).reshape(128, KT_ * fc)
            for f0, fc in zip(f_offs, CHUNKS)
        ]
        return np.concatenate(parts, axis=1)

    in_maps = []
    for e in range(n_experts):
        seg = s_tok[starts[e] : starts[e] + counts[e]]
        xt = np.zeros((KT_, 128, C), BF)
        xt.reshape(d_model, C)[:, : counts[e]] = xb[seg].T
        xts = np.ascontiguousarray(xt.transpose(1, 0, 2)).reshape(128, KT_ * C)
        w3s = np.ascontiguousarray(
            w3[e].reshape(D_FF // 128, 128, d_model).transpose(1, 0, 2)
        ).reshape(128, (D_FF // 128) * d_model)
        in_maps.append({
            "xt": xts,
            "w1": chunked_w12(w1[e]),
            "w2": chunked_w12(w2[e]),
            "w3": w3s,
        })

    nc = _get_program(C)
    res = _run(nc, in_maps, trace=_trace)

    y = np.empty((A, d_model), np.float32)
    for e in range(n_experts):
        ot = res.results[e]["ot"]
        y[starts[e] : starts[e] + counts[e]] = ot[:, : counts[e]].T
    y *= s_w[:, None]
    y_orig = np.empty_like(y)
    y_orig[order] = y
    out = y_orig.reshape(n_tokens, top_k, d_model).sum(axis=1, dtype=np.float32)
    if _trace:
        return out.astype(np.float32, copy=False), res
    return out.astype(np.float32, copy=False)


# revision 16
# speedup vs baseline: 1.0000x; 1.0000x over previous
"""Batched MoE (top-2, 8 experts) on 8 Trainium2 NeuronCores.

Strategy: expert-parallel — core e owns expert e's weights (w1/w2/w3) and
processes the tokens routed to it. Routing (sort by expert / capacity
padding) and the combine (weighting by gate prob + scatter-add over top-k)
are cheap O(tokens) index ops done on host; all matmul FLOPs run on device.

Device dataflow per core (capacity C columns, zero-padded):
    xt  = X_e^T               [1024, C]   (d on partitions)
    GT  = w1^T @ xt           [4096, C]   lhsT = w1 tiles (natural layout)
    VT  = w2^T @ xt           [4096, C]
    HT  = silu(GT) * VT       [4096, C]
    OT  = w3^T @ HT           [1024, C]   lhsT = w3 tiles (natural layout)

Everything is bf16 (PSUM/OT accumulation in fp32): same PE rate as
float32r (1 col/cycle) but half the HBM traffic, and bf16 LDWEIGHTS (FWL)
hides under the previous matmul's streaming. Matmul loops are
column-chunk-outer / k-inner so consecutive matmuls always carry different
weight tiles (a same-cell weight reload stalls the PE).

All tensors ride ONE merged DMA per (matrix, chunk) using a
"(k p) f -> p k f" partition-major rearrange: ~1MB transfers keep the DMA
queue near line rate, and the short first/last chunks (f=256) shrink the
DMA critical path at the ramp (so HAM un-throttles early) and the
phase-B/store drain at the tail. OT is stored as bf16; the final
accumulation's DVE add does the fp32->bf16 cast for free.
"""

import numpy as np

N_EXPERTS = 8
D_MODEL = 1024
D_FF = 4096
# first/last chunks short: fast ramp, short tail
CHUNKS = [256, 512, 512, 512, 512, 512, 512, 512, 256]
KT = D_MODEL // 128     # 8 k-tiles (contraction d)
MT = D_MODEL // 128     # 8 output d-tiles

_program_cache = {}


def _col_chunks(C):
    """Split C columns into <=512 pieces (PSUM bank = 512 fp32)."""
    if C <= 512:
        return [(0, C)]
    n = (C + 511) // 512
    base = C // n
    rem = C - base * n
    out = []
    off = 0
    for i in range(n):
        sz = base + (1 if i < rem else 0)
        out.append((off, sz))
        off += sz
    return out


def _build_program(C):
    import concourse.bacc as bacc
    import concourse.mybir as mybir
    from concourse.tile import TileContext

    BF16 = mybir.dt.bfloat16
    F32 = mybir.dt.float32
    SILU = mybir.ActivationFunctionType.Silu
    ccs = _col_chunks(C)

    nc = bacc.Bacc()
    xt_d = nc.declare_dram_parameter("xt", [D_MODEL, C], BF16, isOutput=False)
    w1_d = nc.declare_dram_parameter("w1", [D_MODEL, D_FF], BF16, isOutput=False)
    w2_d = nc.declare_dram_parameter("w2", [D_MODEL, D_FF], BF16, isOutput=False)
    w3_d = nc.declare_dram_parameter("w3", [D_FF, D_MODEL], BF16, isOutput=False)
    ot_d = nc.declare_dram_parameter("ot", [D_MODEL, C], BF16, isOutput=True)

    # partition-major views: one merged DMA per (matrix, chunk)
    xt_r = xt_d.rearrange("(k p) c -> p k c", p=128)
    w1_r = w1_d.rearrange("(k p) f -> p k f", p=128)
    w2_r = w2_d.rearrange("(k p) f -> p k f", p=128)
    w3_r = w3_d.rearrange("(j p) d -> p j d", p=128)
    ot_r = ot_d.rearrange("(m p) c -> m p c", p=128)

    NCH = len(CHUNKS)
    f_offs = [sum(CHUNKS[:i]) for i in range(NCH)]

    with TileContext(nc) as tc:
        with (
            tc.tile_pool(name="xtp", bufs=1) as xt_pool,
            tc.tile_pool(name="w12", bufs=2) as w12_pool,
            tc.tile_pool(name="w3p", bufs=2) as w3_pool,
            tc.tile_pool(name="htp", bufs=2) as ht_pool,
            tc.tile_pool(name="otp", bufs=1) as ot_pool,
            tc.tile_pool(name="tmp", bufs=4) as tmp_pool,
            tc.tile_pool(name="pg", bufs=2, space="PSUM") as pg_pool,
            tc.tile_pool(name="pv", bufs=2, space="PSUM") as pv_pool,
            tc.tile_pool(name="po", bufs=4, space="PSUM") as po_pool,
        ):
            ot_sb = [
                ot_pool.tile([128, C], F32, tag=f"ot{m}", name=f"ot{m}")
                for m in range(MT)
            ]
            ot_bf = [
                ot_pool.tile([128, C], BF16, tag=f"otb{m}", name=f"otb{m}")
                for m in range(MT)
            ]

            # HAM warm-up: a dependency-free matmul burst fills the
            # ~5us the PE would otherwise idle waiting for the first weight
            # DMAs, so the clock gate is at 2.4GHz when real work arrives.
            dmy = tmp_pool.tile([128, 264], BF16, tag="dmy", name="dmy")
            nc.vector.memset(dmy[:], 0.0)
            pw = po_pool.tile(
                [128, 264], F32, tag="po", name="pw", padded_shape=[128, 512]
            )
            for i in range(16):
                nc.tensor.matmul(
                    out=pw[:],
                    lhsT=dmy[:, 128 * (i % 2) : 128 * (i % 2) + 128],
                    rhs=dmy[:],
                    start=(i == 0),
                    stop=(i == 15),
                )

            # xt streams in two column-halves so chunk 0's first column-chunk
            # can compute while the second half is still in flight
            xt_sb = xt_pool.tile([128, KT, C], BF16, tag="xt", name="xt")
            cc0l = ccs[0][1]
            nc.sync.dma_start(
                out=xt_sb[:, :, 0:cc0l], in_=xt_r[:, :, 0:cc0l]
            )

            def load_w1(ch):
                f0, fc = f_offs[ch], CHUNKS[ch]
                t1 = w12_pool.tile(
                    [128, KT, fc], BF16, tag="w1c", name=f"w1c{ch}",
                    padded_shape=[128, KT, 512],
                )
                nc.sync.dma_start(out=t1[:], in_=w1_r[:, :, f0 : f0 + fc])
                return t1

            def load_w2(ch):
                f0, fc = f_offs[ch], CHUNKS[ch]
                t2 = w12_pool.tile(
                    [128, KT, fc], BF16, tag="w2c", name=f"w2c{ch}",
                    padded_shape=[128, KT, 512],
                )
                nc.sync.dma_start(out=t2[:], in_=w2_r[:, :, f0 : f0 + fc])
                return t2

            def load_w3(ch):
                f0, fc = f_offs[ch], CHUNKS[ch]
                jt = fc // 128
                t3 = w3_pool.tile(
                    [128, jt, D_MODEL], BF16, tag="w3c", name=f"w3c{ch}",
                    padded_shape=[128, 4, D_MODEL],
                )
                j0 = f0 // 128
                nc.sync.dma_start(out=t3[:], in_=w3_r[:, j0 : j0 + jt, :])
                return t3

            def phase_a(ch, w1c, w2c):
                """GT/VT matmuls + silu*mul epilogue -> HT tiles for a chunk.

                For chunk 0 both column-chunks' G groups run before the V
                groups so the PE has w1-only work while w2's DMA is still in
                flight at the ramp."""
                jt = CHUNKS[ch] // 128
                hts = []
                for jj in range(jt):
                    ht_t = ht_pool.tile(
                        [128, C], BF16, tag=f"ht{jj % 4}", name=f"ht{jj}"
                    )
                    js = slice(jj * 128, (jj + 1) * 128)
                    pgs, pvs = [], []
                    for c0, cl in ccs:
                        pgs.append(pg_pool.tile(
                            [128, cl], F32, tag="pg", name="pg",
                            padded_shape=[128, 512],
                        ))
                        pvs.append(pv_pool.tile(
                            [128, cl], F32, tag="pv", name="pv",
                            padded_shape=[128, 512],
                        ))

                    def g_group(i):
                        c0, cl = ccs[i]
                        for k in range(KT):
                            nc.tensor.matmul(
                                out=pgs[i][:],
                                lhsT=w1c[:, k, js],
                                rhs=xt_sb[:, k, c0 : c0 + cl],
                                start=(k == 0),
                                stop=(k == KT - 1),
                            )

                    def v_group(i):
                        c0, cl = ccs[i]
                        for k in range(KT):
                            nc.tensor.matmul(
                                out=pvs[i][:],
                                lhsT=w2c[:, k, js],
                                rhs=xt_sb[:, k, c0 : c0 + cl],
                                start=(k == 0),
                                stop=(k == KT - 1),
                            )

                    def epilogue(i):
                        c0, cl = ccs[i]
                        cs = slice(c0, c0 + cl)
                        st = tmp_pool.tile([128, cl], F32, tag="silu", name="st")
                        nc.scalar.activation(st[:], pgs[i][:], SILU)
                        nc.vector.tensor_mul(
                            out=ht_t[:, cs], in0=st[:], in1=pvs[i][:]
                        )

                    for i in range(len(ccs)):
                        g_group(i)
                        v_group(i)
                        epilogue(i)
                    hts.append(ht_t)
                return hts

            def phase_a0(w1c, w2c):
                """Chunk 0, ramp-ordered: process column-chunk 0 entirely
                (G for all jj, then V+epilogue) before touching column-chunk
                1, matching the DMA arrival order xt_h0, w1c0, w2c0, xt_h1."""
                jt = CHUNKS[0] // 128
                hts = [
                    ht_pool.tile([128, C], BF16, tag=f"ht{jj % 4}", name=f"ht{jj}")
                    for jj in range(jt)
                ]
                for i, (c0, cl) in enumerate(ccs):
                    cs = slice(c0, c0 + cl)
                    pgs = []
                    for jj in range(jt):
                        js = slice(jj * 128, (jj + 1) * 128)
                        pg = pg_pool.tile(
                            [128, cl], F32, tag="pg", name="pg",
                            padded_shape=[128, 512],
                        )
                        for k in range(KT):
                            nc.tensor.matmul(
                                out=pg[:],
                                lhsT=w1c[:, k, js],
                                rhs=xt_sb[:, k, cs],
                                start=(k == 0),
                                stop=(k == KT - 1),
                            )
                        pgs.append(pg)
                    for jj in range(jt):
                        js = slice(jj * 128, (jj + 1) * 128)
                        pv = pv_pool.tile(
                            [128, cl], F32, tag="pv", name="pv",
                            padded_shape=[128, 512],
                        )
                        for k in range(KT):
                            nc.tensor.matmul(
                                out=pv[:],
                                lhsT=w2c[:, k, js],
                                rhs=xt_sb[:, k, cs],
                                start=(k == 0),
                                stop=(k == KT - 1),
                            )
                        st = tmp_pool.tile([128, cl], F32, tag="silu", name="st")
                        nc.scalar.activation(st[:], pgs[jj][:], SILU)
                        nc.vector.tensor_mul(
                            out=hts[jj][:, cs], in0=st[:], in1=pv[:]
                        )
                return hts

            def phase_b_m(ch, w3c, hts, m):
                """OT partial accumulation for one output d-tile of a chunk."""
                jt = len(hts)
                ms = slice(m * 128, (m + 1) * 128)
                for c0, cl in ccs:
                    cs = slice(c0, c0 + cl)
                    po = po_pool.tile(
                        [128, cl], F32, tag="po", name="po",
                        padded_shape=[128, 512],
                    )
                    for jj in range(jt):
                        nc.tensor.matmul(
                            out=po[:],
                            lhsT=w3c[:, jj, ms],
                            rhs=hts[jj][:, cs],
                            start=(jj == 0),
                            stop=(jj == jt - 1),
                        )
                    if ch == 0:
                        nc.vector.tensor_copy(out=ot_sb[m][:, cs], in_=po[:])
                    elif ch == NCH - 1:
                        # final accumulation writes the bf16 store tile (DVE
                        # converts on output) so the OT store DMA is half-size
                        nc.vector.tensor_add(
                            out=ot_bf[m][:, cs], in0=ot_sb[m][:, cs], in1=po[:]
                        )
                    else:
                        nc.vector.tensor_add(
                            out=ot_sb[m][:, cs], in0=ot_sb[m][:, cs], in1=po[:]
                        )

            def phase_b(ch, w3c, hts):
                for m in range(MT):
                    phase_b_m(ch, w3c, hts, m)

            # software pipeline: B(ch-1) issues after A(ch) so phase B never
            # stalls the PE on the ACT/DVE epilogue producing its HT input.
            # DMAs are emitted in exact PE consumption order. Only the last
            # (short) B pass drains at the end, interleaved m-wise with the
            # (bf16, half-size) OT store DMAs.
            w1c, w2c = load_w1(0), load_w2(0)
            if len(ccs) > 1:
                nc.sync.dma_start(
                    out=xt_sb[:, :, cc0l:C], in_=xt_r[:, :, cc0l:C]
                )
            hts_prev = phase_a0(w1c, w2c)
            w3_prev = None
            hts_m2 = None
            for ch in range(1, NCH):
                w1c, w2c = load_w1(ch), load_w2(ch)
                w3_prev = load_w3(ch - 1)
                hts = phase_a(ch, w1c, w2c)
                if ch < NCH - 1:
                    phase_b(ch - 1, w3_prev, hts_prev)
                else:
                    hts_m2 = hts_prev  # B(NCH-2) merges into the final pass
                hts_prev = hts
            w3_last = load_w3(NCH - 1)
            # final pass: one PSUM group accumulates BOTH remaining chunks'
            # contributions per (m, column-chunk) -> half the endgame DVE adds
            for m in range(MT):
                ms = slice(m * 128, (m + 1) * 128)
                for c0, cl in ccs:
                    cs = slice(c0, c0 + cl)
                    po = po_pool.tile(
                        [128, cl], F32, tag="po", name="po",
                        padded_shape=[128, 512],
                    )
                    groups = [(w3_prev, hts_m2), (w3_last, hts_prev)]
                    n_tot = sum(len(h) for _, h in groups)
                    idx = 0
                    for w3c, hts_g in groups:
                        for jj in range(len(hts_g)):
                            nc.tensor.matmul(
                                out=po[:],
                                lhsT=w3c[:, jj, ms],
                                rhs=hts_g[jj][:, cs],
                                start=(idx == 0),
                                stop=(idx == n_tot - 1),
                            )
                            idx += 1
                    nc.vector.tensor_add(
                        out=ot_bf[m][:, cs], in0=ot_sb[m][:, cs], in1=po[:]
                    )
                nc.sync.dma_start(out=ot_r[m], in_=ot_bf[m][:])

    nc.compile()
    return nc


def _get_program(C):
    if C not in _program_cache:
        _program_cache[C] = _build_program(C)
    return _program_cache[C]


def _run(nc, in_maps, trace=False):
    import time

    from concourse.bass_utils import run_bass_kernel_spmd

    last = None
    for attempt in range(4):
        try:
            return run_bass_kernel_spmd(
                nc, in_maps, list(range(N_EXPERTS)), trace=trace
            )
        except Exception as e:  # stale device state from a prior crashed run
            last = e
            time.sleep(10 * (attempt + 1))
            try:  # poke the runtime with a trivial op to clear/verify state
                import jax
                import jax.numpy as jnp

                jnp.add(jnp.ones((8, 8)), 1.0).block_until_ready()
            except Exception:
                pass
    raise last


def kernel(x, expert_indices, expert_weights, w1, w2, w3, _trace=False):
    import ml_dtypes

    BF = ml_dtypes.bfloat16
    x = np.ascontiguousarray(np.asarray(x, dtype=np.float32))
    expert_indices = np.asarray(expert_indices)
    expert_weights = np.asarray(expert_weights, dtype=np.float32)
    w1 = np.asarray(w1, dtype=np.float32).astype(BF)
    w2 = np.asarray(w2, dtype=np.float32).astype(BF)
    w3 = np.asarray(w3, dtype=np.float32).astype(BF)

    n_tokens, d_model = x.shape
    top_k = expert_indices.shape[1]
    n_experts = w1.shape[0]
    A = n_tokens * top_k

    flat_e = expert_indices.reshape(-1).astype(np.int64)
    flat_w = expert_weights.reshape(-1)
    tok_idx = np.repeat(np.arange(n_tokens), top_k)
    order = np.argsort(flat_e, kind="stable")
    s_tok = tok_idx[order]
    s_w = flat_w[order]
    counts = np.bincount(flat_e, minlength=n_experts)
    starts = np.concatenate([[0], np.cumsum(counts)[:-1]])

    C = int(counts.max())
    C = max(256, -(-C // 8) * 8)  # round up to multiple of 8 (16B bf16 rows)

    xb = x.astype(BF)
    xt = np.zeros((n_experts, d_model, C), BF)
    for e in range(n_experts):
        seg = s_tok[starts[e] : starts[e] + counts[e]]
        xt[e, :, : counts[e]] = xb[seg].T

    nc = _get_program(C)
    in_maps = [
        {"xt": xt[e], "w1": w1[e], "w2": w2[e], "w3": w3[e]}
        for e in range(n_experts)
    ]
    res = _run(nc, in_maps, trace=_trace)

    y = np.empty((A, d_model), np.float32)
    for e in range(n_experts):
        ot = res.results[e]["ot"]
        y[starts[e] : starts[e] + counts[e]] = ot[:, : counts[e]].T
    y *= s_w[:, None]
    y_orig = np.empty_like(y)
    y_orig[order] = y
    out = y_orig.reshape(n_tokens, top_k, d_model).sum(axis=1, dtype=np.float32)
    if _trace:
        return out.astype(np.float32, copy=False), res
    return out.astype(np.float32, copy=False)


# revision 17
# speedup vs baseline: 1.0029x; 1.0029x over previous
"""Batched MoE (top-2, 8 experts) on 8 Trainium2 NeuronCores.

Strategy: expert-parallel — core e owns expert e's weights (w1/w2/w3) and
processes the tokens routed to it. Routing (sort by expert / capacity
padding) and the combine (weighting by gate prob + scatter-add over top-k)
are cheap O(tokens) index ops done on host; all matmul FLOPs run on device.

Device dataflow per core (capacity C columns, zero-padded):
    xt  = X_e^T               [1024, C]   (d on partitions)
    GT  = w1^T @ xt           [4096, C]   lhsT = w1 tiles (natural layout)
    VT  = w2^T @ xt           [4096, C]
    HT  = silu(GT) * VT       [4096, C]
    OT  = w3^T @ HT           [1024, C]   lhsT = w3 tiles (natural layout)

Everything is bf16 (PSUM/OT accumulation in fp32): same PE rate as
float32r (1 col/cycle) but half the HBM traffic, and bf16 LDWEIGHTS (FWL)
hides under the previous matmul's streaming. Matmul loops are
column-chunk-outer / k-inner so consecutive matmuls always carry different
weight tiles (a same-cell weight reload stalls the PE).

All tensors ride ONE merged DMA per (matrix, chunk) using a
"(k p) f -> p k f" partition-major rearrange: ~1MB transfers keep the DMA
queue near line rate, and the short first/last chunks (f=256) shrink the
DMA critical path at the ramp (so HAM un-throttles early) and the
phase-B/store drain at the tail. OT is stored as bf16; the final
accumulation's DVE add does the fp32->bf16 cast for free.
"""

import numpy as np

N_EXPERTS = 8
D_MODEL = 1024
D_FF = 4096
# first/last chunks short: fast ramp, short tail
CHUNKS = [256, 512, 512, 512, 512, 512, 512, 512, 256]
KT = D_MODEL // 128     # 8 k-tiles (contraction d)
MT = D_MODEL // 128     # 8 output d-tiles

_program_cache = {}


def _col_chunks(C):
    """Split C columns into <=512 pieces (PSUM bank = 512 fp32)."""
    if C <= 512:
        return [(0, C)]
    n = (C + 511) // 512
    base = C // n
    rem = C - base * n
    out = []
    off = 0
    for i in range(n):
        sz = base + (1 if i < rem else 0)
        out.append((off, sz))
        off += sz
    return out


def _build_program(C):
    import concourse.bacc as bacc
    import concourse.mybir as mybir
    from concourse.tile import TileContext

    BF16 = mybir.dt.bfloat16
    F32 = mybir.dt.float32
    SILU = mybir.ActivationFunctionType.Silu
    ccs = _col_chunks(C)

    nc = bacc.Bacc()
    xt_d = nc.declare_dram_parameter("xt", [D_MODEL, C], BF16, isOutput=False)
    w1_d = nc.declare_dram_parameter("w1", [D_MODEL, D_FF], BF16, isOutput=False)
    w2_d = nc.declare_dram_parameter("w2", [D_MODEL, D_FF], BF16, isOutput=False)
    w3_d = nc.declare_dram_parameter("w3", [D_FF, D_MODEL], BF16, isOutput=False)
    ot_d = nc.declare_dram_parameter("ot", [D_MODEL, C], BF16, isOutput=True)

    # partition-major views: one merged DMA per (matrix, chunk)
    xt_r = xt_d.rearrange("(k p) c -> p k c", p=128)
    w1_r = w1_d.rearrange("(k p) f -> p k f", p=128)
    w2_r = w2_d.rearrange("(k p) f -> p k f", p=128)
    w3_r = w3_d.rearrange("(j p) d -> p j d", p=128)
    ot_r = ot_d.rearrange("(m p) c -> m p c", p=128)

    NCH = len(CHUNKS)
    f_offs = [sum(CHUNKS[:i]) for i in range(NCH)]

    with TileContext(nc) as tc:
        with (
            tc.tile_pool(name="xtp", bufs=1) as xt_pool,
            tc.tile_pool(name="w12", bufs=2) as w12_pool,
            tc.tile_pool(name="w3p", bufs=2) as w3_pool,
            tc.tile_pool(name="htp", bufs=2) as ht_pool,
            tc.tile_pool(name="otp", bufs=1) as ot_pool,
            tc.tile_pool(name="tmp", bufs=4) as tmp_pool,
            tc.tile_pool(name="pg", bufs=2, space="PSUM") as pg_pool,
            tc.tile_pool(name="pv", bufs=2, space="PSUM") as pv_pool,
            tc.tile_pool(name="po", bufs=4, space="PSUM") as po_pool,
        ):
            ot_sb = [
                ot_pool.tile([128, C], F32, tag=f"ot{m}", name=f"ot{m}")
                for m in range(MT)
            ]
            ot_bf = [
                ot_pool.tile([128, C], BF16, tag=f"otb{m}", name=f"otb{m}")
                for m in range(MT)
            ]

            # HAM warm-up: a dependency-free matmul burst fills the
            # ~5us the PE would otherwise idle waiting for the first weight
            # DMAs, so the clock gate is at 2.4GHz when real work arrives.
            dmy = tmp_pool.tile([128, 264], BF16, tag="dmy", name="dmy")
            nc.vector.memset(dmy[:], 0.0)
            pw = po_pool.tile(
                [128, 264], F32, tag="po", name="pw", padded_shape=[128, 512]
            )
            for i in range(10):
                nc.tensor.matmul(
                    out=pw[:],
                    lhsT=dmy[:, 128 * (i % 2) : 128 * (i % 2) + 128],
                    rhs=dmy[:],
                    start=(i == 0),
                    stop=(i == 9),
                )

            # xt streams in two column-halves so chunk 0's first column-chunk
            # can compute while the second half is still in flight
            xt_sb = xt_pool.tile([128, KT, C], BF16, tag="xt", name="xt")
            cc0l = ccs[0][1]
            nc.sync.dma_start(
                out=xt_sb[:, :, 0:cc0l], in_=xt_r[:, :, 0:cc0l]
            )

            def load_w1(ch):
                f0, fc = f_offs[ch], CHUNKS[ch]
                t1 = w12_pool.tile(
                    [128, KT, fc], BF16, tag="w1c", name=f"w1c{ch}",
                    padded_shape=[128, KT, 512],
                )
                nc.sync.dma_start(out=t1[:], in_=w1_r[:, :, f0 : f0 + fc])
                return t1

            def load_w2(ch):
                f0, fc = f_offs[ch], CHUNKS[ch]
                t2 = w12_pool.tile(
                    [128, KT, fc], BF16, tag="w2c", name=f"w2c{ch}",
                    padded_shape=[128, KT, 512],
                )
                nc.sync.dma_start(out=t2[:], in_=w2_r[:, :, f0 : f0 + fc])
                return t2

            def load_w3(ch):
                f0, fc = f_offs[ch], CHUNKS[ch]
                jt = fc // 128
                t3 = w3_pool.tile(
                    [128, jt, D_MODEL], BF16, tag="w3c", name=f"w3c{ch}",
                    padded_shape=[128, 4, D_MODEL],
                )
                j0 = f0 // 128
                nc.sync.dma_start(out=t3[:], in_=w3_r[:, j0 : j0 + jt, :])
                return t3

            def phase_a(ch, w1c, w2c):
                """GT/VT matmuls + silu*mul epilogue -> HT tiles for a chunk.

                For chunk 0 both column-chunks' G groups run before the V
                groups so the PE has w1-only work while w2's DMA is still in
                flight at the ramp."""
                jt = CHUNKS[ch] // 128
                hts = []
                for jj in range(jt):
                    ht_t = ht_pool.tile(
                        [128, C], BF16, tag=f"ht{jj % 4}", name=f"ht{jj}"
                    )
                    js = slice(jj * 128, (jj + 1) * 128)
                    pgs, pvs = [], []
                    for c0, cl in ccs:
                        pgs.append(pg_pool.tile(
                            [128, cl], F32, tag="pg", name="pg",
                            padded_shape=[128, 512],
                        ))
                        pvs.append(pv_pool.tile(
                            [128, cl], F32, tag="pv", name="pv",
                            padded_shape=[128, 512],
                        ))

                    def g_group(i):
                        c0, cl = ccs[i]
                        for k in range(KT):
                            nc.tensor.matmul(
                                out=pgs[i][:],
                                lhsT=w1c[:, k, js],
                                rhs=xt_sb[:, k, c0 : c0 + cl],
                                start=(k == 0),
                                stop=(k == KT - 1),
                            )

                    def v_group(i):
                        c0, cl = ccs[i]
                        for k in range(KT):
                            nc.tensor.matmul(
                                out=pvs[i][:],
                                lhsT=w2c[:, k, js],
                                rhs=xt_sb[:, k, c0 : c0 + cl],
                                start=(k == 0),
                                stop=(k == KT - 1),
                            )

                    def epilogue(i):
                        c0, cl = ccs[i]
                        cs = slice(c0, c0 + cl)
                        st = tmp_pool.tile([128, cl], F32, tag="silu", name="st")
                        nc.scalar.activation(st[:], pgs[i][:], SILU)
                        nc.vector.tensor_mul(
                            out=ht_t[:, cs], in0=st[:], in1=pvs[i][:]
                        )

                    for i in range(len(ccs)):
                        g_group(i)
                        v_group(i)
                        epilogue(i)
                    hts.append(ht_t)
                return hts

            def phase_a0(w1c, w2c):
                """Chunk 0, ramp-ordered: process column-chunk 0 entirely
                (G for all jj, then V+epilogue) before touching column-chunk
                1, matching the DMA arrival order xt_h0, w1c0, w2c0, xt_h1."""
                jt = CHUNKS[0] // 128
                hts = [
                    ht_pool.tile([128, C], BF16, tag=f"ht{jj % 4}", name=f"ht{jj}")
                    for jj in range(jt)
                ]
                for i, (c0, cl) in enumerate(ccs):
                    cs = slice(c0, c0 + cl)
                    pgs = []
                    for jj in range(jt):
                        js = slice(jj * 128, (jj + 1) * 128)
                        pg = pg_pool.tile(
                            [128, cl], F32, tag="pg", name="pg",
                            padded_shape=[128, 512],
                        )
                        for k in range(KT):
                            nc.tensor.matmul(
                                out=pg[:],
                                lhsT=w1c[:, k, js],
                                rhs=xt_sb[:, k, cs],
                                start=(k == 0),
                                stop=(k == KT - 1),
                            )
                        pgs.append(pg)
                    for jj in range(jt):
                        js = slice(jj * 128, (jj + 1) * 128)
                        pv = pv_pool.tile(
                            [128, cl], F32, tag="pv", name="pv",
                            padded_shape=[128, 512],
                        )
                        for k in range(KT):
                            nc.tensor.matmul(
                                out=pv[:],
                                lhsT=w2c[:, k, js],
                                rhs=xt_sb[:, k, cs],
                                start=(k == 0),
                                stop=(k == KT - 1),
                            )
                        st = tmp_pool.tile([128, cl], F32, tag="silu", name="st")
                        nc.scalar.activation(st[:], pgs[jj][:], SILU)
                        nc.vector.tensor_mul(
                            out=hts[jj][:, cs], in0=st[:], in1=pv[:]
                        )
                return hts

            def phase_b_m(ch, w3c, hts, m):
                """OT partial accumulation for one output d-tile of a chunk."""
                jt = len(hts)
                ms = slice(m * 128, (m + 1) * 128)
                for c0, cl in ccs:
                    cs = slice(c0, c0 + cl)
                    po = po_pool.tile(
                        [128, cl], F32, tag="po", name="po",
                        padded_shape=[128, 512],
                    )
                    for jj in range(jt):
                        nc.tensor.matmul(
                            out=po[:],
                            lhsT=w3c[:, jj, ms],
                            rhs=hts[jj][:, cs],
                            start=(jj == 0),
                            stop=(jj == jt - 1),
                        )
                    if ch == 0:
                        nc.vector.tensor_copy(out=ot_sb[m][:, cs], in_=po[:])
                    elif ch == NCH - 1:
                        # final accumulation writes the bf16 store tile (DVE
                        # converts on output) so the OT store DMA is half-size
                        nc.vector.tensor_add(
                            out=ot_bf[m][:, cs], in0=ot_sb[m][:, cs], in1=po[:]
                        )
                    else:
                        nc.vector.tensor_add(
                            out=ot_sb[m][:, cs], in0=ot_sb[m][:, cs], in1=po[:]
                        )

            def phase_b(ch, w3c, hts):
                for m in range(MT):
                    phase_b_m(ch, w3c, hts, m)

            # software pipeline: B(ch-1) issues after A(ch) so phase B never
            # stalls the PE on the ACT/DVE epilogue producing its HT input.
            # DMAs are emitted in exact PE consumption order. Only the last
            # (short) B pass drains at the end, interleaved m-wise with the
            # (bf16, half-size) OT store DMAs.
            w1c, w2c = load_w1(0), load_w2(0)
            if len(ccs) > 1:
                nc.sync.dma_start(
                    out=xt_sb[:, :, cc0l:C], in_=xt_r[:, :, cc0l:C]
                )
            hts_prev = phase_a0(w1c, w2c)
            w3_prev = None
            hts_m2 = None
            for ch in range(1, NCH):
                w1c, w2c = load_w1(ch), load_w2(ch)
                w3_prev = load_w3(ch - 1)
                hts = phase_a(ch, w1c, w2c)
                if ch < NCH - 1:
                    phase_b(ch - 1, w3_prev, hts_prev)
                else:
                    hts_m2 = hts_prev  # B(NCH-2) merges into the final pass
                hts_prev = hts
            w3_last = load_w3(NCH - 1)
            # final pass: one PSUM group accumulates BOTH remaining chunks'
            # contributions per (m, column-chunk) -> half the endgame DVE adds
            for m in range(MT):
                ms = slice(m * 128, (m + 1) * 128)
                for c0, cl in ccs:
                    cs = slice(c0, c0 + cl)
                    po = po_pool.tile(
                        [128, cl], F32, tag="po", name="po",
                        padded_shape=[128, 512],
                    )
                    groups = [(w3_prev, hts_m2), (w3_last, hts_prev)]
                    n_tot = sum(len(h) for _, h in groups)
                    idx = 0
                    for w3c, hts_g in groups:
                        for jj in range(len(hts_g)):
                            nc.tensor.matmul(
                                out=po[:],
                                lhsT=w3c[:, jj, ms],
                                rhs=hts_g[jj][:, cs],
                                start=(idx == 0),
                                stop=(idx == n_tot - 1),
                            )
                            idx += 1
                    nc.vector.tensor_add(
                        out=ot_bf[m][:, cs], in0=ot_sb[m][:, cs], in1=po[:]
                    )
                nc.sync.dma_start(out=ot_r[m], in_=ot_bf[m][:])

    nc.compile()
    return nc


def _get_program(C):
    if C not in _program_cache:
        _program_cache[C] = _build_program(C)
    return _program_cache[C]


def _run(nc, in_maps, trace=False):
    import time

    from concourse.bass_utils import run_bass_kernel_spmd

    last = None
    for attempt in range(4):
        try:
            return run_bass_kernel_spmd(
                nc, in_maps, list(range(N_EXPERTS)), trace=trace
            )
        except Exception as e:  # stale device state from a prior crashed run
            last = e
            time.sleep(10 * (attempt + 1))
            try:  # poke the runtime with a trivial op to clear/verify state
                import jax
                import jax.numpy as jnp

                jnp.add(jnp.ones((8, 8)), 1.0).block_until_ready()
            except Exception:
                pass
    raise last


def kernel(x, expert_indices, expert_weights, w1, w2, w3, _trace=False):
    import ml_dtypes

    BF = ml_dtypes.bfloat16
    x = np.ascontiguousarray(np.asarray(x, dtype=np.float32))
    expert_indices = np.asarray(expert_indices)
    expert_weights = np.asarray(expert_weights, dtype=np.float32)
    w1 = np.asarray(w1, dtype=np.float32).astype(BF)
    w2 = np.asarray(w2, dtype=np.float32).astype(BF)
    w3 = np.asarray(w3, dtype=np.float32).astype(BF)

    n_tokens, d_model = x.shape
    top_k = expert_indices.shape[1]
    n_experts = w1.shape[0]
    A = n_tokens * top_k

    flat_e = expert_indices.reshape(-1).astype(np.int64)
    flat_w = expert_weights.reshape(-1)
    tok_idx = np.repeat(np.arange(n_tokens), top_k)
    order = np.argsort(flat_e, kind="stable")
    s_tok = tok_idx[order]
    s_w = flat_w[order]
    counts = np.bincount(flat_e, minlength=n_experts)
    starts = np.concatenate([[0], np.cumsum(counts)[:-1]])

    C = int(counts.max())
    C = max(256, -(-C // 8) * 8)  # round up to multiple of 8 (16B bf16 rows)

    xb = x.astype(BF)
    xt = np.zeros((n_experts, d_model, C), BF)
    for e in range(n_experts):
        seg = s_tok[starts[e] : starts[e] + counts[e]]
        xt[e, :, : counts[e]] = xb[seg].T

    nc = _get_program(C)
    in_maps = [
        {"xt": xt[e], "w1": w1[e], "w2": w2[e], "w3": w3[e]}
        for e in range(n_experts)
    ]
    res = _run(nc, in_maps, trace=_trace)

    y = np.empty((A, d_model), np.float32)
    for e in range(n_experts):
        ot = res.results[e]["ot"]
        y[starts[e] : starts[e] + counts[e]] = ot[:, : counts[e]].T
    y *= s_w[:, None]
    y_orig = np.empty_like(y)
    y_orig[order] = y
    out = y_orig.reshape(n_tokens, top_k, d_model).sum(axis=1, dtype=np.float32)
    if _trace:
        return out.astype(np.float32, copy=False), res
    return out.astype(np.float32, copy=False)
